# revision 13
# baseline (speedup 1.0000x reference)
"""Trainium2 Bass kernel for nn_CNNFromScratch (dense 1-D CNN + MLP head).

Strategy
--------
Pure data parallelism: the batch axis (8192) is split across 8 NeuronCores
(1024 samples each); conv kernels and MLP weights are replicated.

Per core, everything is expressed as TensorE matmuls with the contraction
(input channels x taps) on the partition axis:

  - x is pre-transposed on host to (C=512, tile, W=20, bt) and cast to bf16,
    so each (tile, c-chunk) loads as one DMA whose per-partition run is a
    single 20*bt*2-byte contiguous block (full HBM streaming bandwidth).
  - conv_k == sum over taps of  W_tap^T @ x[:, :, w+tap]  accumulated in PSUM.
  - Activations stay on-chip (SBUF, bf16) between layers; layout is
    (C_out partitions, w-major * batch free).
  - conv3's maxpool is fused from PSUM: relu(max(a,b)) == max-then-relu, so
    the per-position h3 buffer and its DVE relus disappear.

Batch is processed in four 256-sample tiles. Tile 0's conv1 is DMA-paced
(chunk-outer order starts the PE on the first 128-channel chunk, with dummy
"clock keeper" matmuls filling the inter-chunk DMA waits so the PE p-state
stays at full frequency); later tiles overlap their x DMA under the previous
tile's conv2/conv3/mlp work. mlp2/mlp3 of tile t are emitted inside tile
t+1's instruction stream so their PSUM->DVE->PE drain chains hide under
real matmul work.

Bulk DMAs are dep-free and rely on per-ring FIFO order for sequential
completion at full bandwidth. Constraints found by measurement: a
dep-waiting dma_start head-of-line blocks its issuing engine's sequencer;
two concurrently-streaming rings share HBM bandwidth (delaying the urgent
early chunks); and the Scalar ring is contended by system traffic
(instruction fetch). So all bulk DMAs ride the private GpSimd ring, in
priority order; only the small w1 (needed first, for PE warmup) goes on
the earlier-starting Scalar ring.

Matmul inputs are bf16 (1 cycle/row on PE), accumulation is fp32 in PSUM.
bt=256 keeps every matmul stream (256 rows) longer than its shadowed
LDWEIGHTS (~214 cyc), so weight loads stay off the critical path.
"""

import sys

sys.path.insert(0, "/opt/trn_rl_repo")

import numpy as np
import ml_dtypes

N_CORES = 8
B, E, W = 8192, 512, 20
BC = B // N_CORES  # samples per core
NT = 4  # batch tiles per core
BT = BC // NT  # samples per tile (256)

BF16 = ml_dtypes.bfloat16

_compiled = {}


def _build():
    import concourse.bass as bass
    from concourse import bacc, mybir
    import concourse.tile as tile

    dt = mybir.dt
    AF = mybir.ActivationFunctionType

    nc = bacc.Bacc(
        "TRN2",
        target_bir_lowering=False,
        debug=False,
        enable_asserts=False,
        num_devices=N_CORES,
    )

    # host packs all multi-chunk weights into 128-partition layouts so each
    # weight tensor is a single DMA
    x_d = nc.dram_tensor("x", (E, NT * W * BT), dt.bfloat16, kind="ExternalInput").ap()
    w1_d = nc.dram_tensor("w1", (128, 4 * 192), dt.bfloat16, kind="ExternalInput").ap()
    w2_d = nc.dram_tensor("w2", (128, 6 * 128), dt.bfloat16, kind="ExternalInput").ap()
    w3_d = nc.dram_tensor("w3", (128, 7 * 256), dt.bfloat16, kind="ExternalInput").ap()
    m1_d = nc.dram_tensor("m1", (128, 8 * 256), dt.bfloat16, kind="ExternalInput").ap()
    m2_d = nc.dram_tensor("m2", (128, 2 * 128), dt.bfloat16, kind="ExternalInput").ap()
    m3_d = nc.dram_tensor("m3", (128, 1), dt.bfloat16, kind="ExternalInput").ap()
    y_d = nc.dram_tensor("y", (1, BC), dt.float32, kind="ExternalOutput").ap()

    with tile.TileContext(nc) as tc:
        with (
            tc.tile_pool(name="sb", bufs=1) as sb,
            tc.tile_pool(name="ps", bufs=8, space="PSUM") as ps,
        ):
            # Bulk DMAs are issued dep-free, in priority order, all from the
            # (otherwise idle) GpSimd engine onto one ring: per-queue FIFO
            # order makes completions sequential at full bandwidth, with no
            # sequencer head-of-line blocking (a dep-waiting dma_start stalls
            # every later instruction on its issuing engine's sequencer).
            def chain(bass_inst):
                return bass_inst

            # ---- weights (resident for the whole kernel) ----
            # w1 loads first: the PE warmup matmuls read it, so the clock
            # ramp starts as soon as the DMA rings are up.
            w1_sb = sb.tile([128, 4 * 192], dt.bfloat16, tag="w1")
            nc.scalar.dma_start(w1_sb[:], w1_d[:, :])

            # Warm the PE clock gate while x streams in (dummy matmuls on the
            # already-loaded w1 tile; results never read) and pull the ACT
            # Relu table load off the critical path.
            warm_ps = ps.tile([128, 512], dt.float32, tag="ps", name="warm_ps")

            def warm(n):
                for _ in range(n):
                    nc.tensor.matmul(
                        warm_ps[0:64, 0:192],
                        w1_sb[:, 0:64],
                        w1_sb[:, 0:192],
                        start=True,
                        stop=True,
                    )

            warm(24)
            warm_act = sb.tile([1, 1], dt.float32, tag="warm_act")
            nc.scalar.activation(warm_act[:], w1_sb[0:1, 0:1], AF.Relu)

            # x chunk DMAs, tile 0 first; bulk weights ride after them
            # (needed from ~conv2 of tile 0); later tiles stream behind.
            x_sb = {}  # (t, q) -> sbuf tile view

            def load_x_tile(ti):
                for q in range(4):
                    t = sb.tile(
                        [128, W * BT],
                        dt.bfloat16,
                        tag=f"x_{q}",
                        bufs=3,
                        name=f"x_{ti}_{q}",
                    )
                    chain(
                        nc.gpsimd.dma_start(
                            t[:],
                            x_d[
                                q * 128 : (q + 1) * 128,
                                ti * W * BT : (ti + 1) * W * BT,
                            ],
                        )
                    )
                    x_sb[(ti, q)] = t

            load_x_tile(0)

            w2_sb = sb.tile([128, 6 * 128], dt.bfloat16, tag="w2")
            chain(nc.gpsimd.dma_start(w2_sb[:], w2_d[:, :]))
            w3_sb = sb.tile([128, 7 * 256], dt.bfloat16, tag="w3")
            chain(nc.gpsimd.dma_start(w3_sb[:], w3_d[:, :]))
            m1_sb = sb.tile([128, 8 * 256], dt.bfloat16, tag="m1")
            chain(nc.gpsimd.dma_start(m1_sb[:], m1_d[:, :]))
            m2_sb = sb.tile([128, 2 * 128], dt.bfloat16, tag="m2")
            chain(nc.gpsimd.dma_start(m2_sb[:], m2_d[:, :]))
            m3_sb = sb.tile([128, 1], dt.bfloat16, tag="m3")
            chain(nc.gpsimd.dma_start(m3_sb[:], m3_d[:, :]))

            for ti in range(1, NT):
                load_x_tile(ti)

            # ---- per-batch-tile pipeline ----
            bt = BT

            def conv1(ti):
                # conv1: (bt,512,20) -> relu -> (bt,64,18)
                # Output positions are packed in pairs: even w on PSUM/SBUF
                # partitions 0-63, odd w on 64-127. The two M=64 accumulation
                # groups land on different PE column groups and execute
                # concurrently (~2x conv1 throughput). Chunk-outer order lets
                # each block start as soon as its c-chunk DMA lands.
                h1 = sb.tile([128, 9 * bt], dt.bfloat16, tag="h1")

                def mms(p1, u, q):
                    for k in range(3):
                        nc.tensor.matmul(
                            p1[0:64, :bt],
                            w1_sb[:, q * 192 + k * 64 : q * 192 + (k + 1) * 64],
                            x_sb[(ti, q)][:, (2 * u + k) * bt : (2 * u + k + 1) * bt],
                            start=(q == 0 and k == 0),
                            stop=(q == 3 and k == 2),
                            skip_group_check=True,
                        )
                        nc.tensor.matmul(
                            p1[64:128, :bt],
                            w1_sb[:, q * 192 + k * 64 : q * 192 + (k + 1) * 64],
                            x_sb[(ti, q)][
                                :, (2 * u + 1 + k) * bt : (2 * u + 2 + k) * bt
                            ],
                            start=(q == 0 and k == 0),
                            stop=(q == 3 and k == 2),
                            skip_group_check=True,
                        )

                # u0..7 accumulate across chunks in 8 PSUM tiles (the 8th
                # rides warm_ps's ring slot); u8's chunk-0..2 matmuls re-read
                # already-resident chunks, filling the wait for chunk 3's DMA
                p1s = [
                    ps.tile([128, 512], dt.float32, tag="ps", name=f"p1_{u}")
                    for u in range(9)
                ]
                for q in range(3):
                    for u in range(8):
                        mms(p1s[u], u, q)
                for q in range(3):
                    mms(p1s[8], 8, q)
                for u in range(9):
                    mms(p1s[u], u, 3)
                for u in range(9):
                    nc.scalar.activation(
                        h1[:, u * bt : (u + 1) * bt], p1s[u][:, :bt], AF.Relu
                    )
                return h1

            def conv2(h1, wlo, whi):
                # conv2: -> relu -> (bt,128,14)
                # h1's parity-split layout lets adjacent taps fuse into one
                # full 128-row contraction: 3 matmuls per position.
                for w in range(wlo, whi):
                    t0 = w // 2
                    blk0 = 0 if w % 2 == 0 else 3
                    p2 = ps.tile([128, 512], dt.float32, tag="ps")
                    for j in range(3):
                        blk = blk0 + j
                        nc.tensor.matmul(
                            p2[:, :bt],
                            w2_sb[:, blk * 128 : (blk + 1) * 128],
                            h1[:, (t0 + j) * bt : (t0 + j + 1) * bt],
                            start=(j == 0),
                            stop=(j == 2),
                        )
                    nc.vector.tensor_relu(
                        h2[:, w * bt : (w + 1) * bt], p2[:, :bt]
                    )

            def conv3_pool_mlp1(ti):
                # conv3: -> (bt,256,8) as two 128-channel halves, pooled
                # straight out of PSUM: max(even,odd) on DVE, relu on ACT
                # (relu(max(a,b)) == max-then-relu).
                pooled = [
                    sb.tile(
                        [128, 4 * bt], dt.bfloat16, tag=f"pool_{m}", name=f"pool_{m}"
                    )
                    for m in range(2)
                ]
                ptmp = sb.tile([128, 4 * bt], dt.bfloat16, tag="ptmp")
                ptm2 = sb.tile([128, 4 * bt], dt.bfloat16, tag="ptm2")
                for m in range(2):
                    for p in range(4):
                        # even position drains via ACT relu (off the critical
                        # path, under the odd position's matmuls); DVE takes
                        # max(relu(even)_sbuf, odd_psum); final ACT relu
                        # completes relu(max(even, odd)).
                        pp = []
                        for w in (2 * p, 2 * p + 1):
                            p3 = ps.tile([128, 512], dt.float32, tag="ps")
                            for k in range(7):
                                nc.tensor.matmul(
                                    p3[:, :bt],
                                    w3_sb[
                                        :, k * 256 + m * 128 : k * 256 + (m + 1) * 128
                                    ],
                                    h2[:, (w + k) * bt : (w + k + 1) * bt],
                                    start=(k == 0),
                                    stop=(k == 6),
                                )
                            pp.append(p3)
                        ecol = ptmp[:, p * bt : (p + 1) * bt]
                        nc.scalar.activation(ecol, pp[0][:, :bt], AF.Relu)
                        mcol = ptm2[:, p * bt : (p + 1) * bt]
                        nc.vector.tensor_max(mcol, ecol, pp[1][:, :bt])
                        nc.scalar.activation(
                            pooled[m][:, p * bt : (p + 1) * bt], mcol, AF.Relu
                        )

                # mlp1: (bt,1024)->(bt,256), f = c*4 + wp. j/q interleaved so
                # the pooled[1]-dependent matmuls start ~8 matmuls after the
                # last conv3 position, covering its max+relu drain.
                g1 = [
                    sb.tile([128, bt], dt.bfloat16, tag=f"g1_{j}", bufs=2, name=f"g1_{j}")
                    for j in range(2)
                ]
                pms = [
                    ps.tile([128, 512], dt.float32, tag="ps", name=f"pm1_{j}")
                    for j in range(2)
                ]
                for q in range(2):
                    for j in range(2):
                        for wp in range(4):
                            nc.tensor.matmul(
                                pms[j][:, :bt],
                                m1_sb[:, (wp * 2 + q) * 256 + j * 128 : (wp * 2 + q) * 256 + (j + 1) * 128],
                                pooled[q][:, wp * bt : (wp + 1) * bt],
                                start=(wp == 0 and q == 0),
                                stop=(wp == 3 and q == 1),
                            )
                for j in range(2):
                    nc.vector.tensor_relu(g1[j][:], pms[j][:, :bt])
                return g1

            def mlp2(g1):
                # mlp2: (bt,256)->(bt,128)
                g2 = sb.tile([128, bt], dt.bfloat16, tag="g2", bufs=2)
                pm = ps.tile([128, 512], dt.float32, tag="ps")
                for q in range(2):
                    nc.tensor.matmul(
                        pm[:, :bt],
                        m2_sb[:, q * 128 : (q + 1) * 128],
                        g1[q][:],
                        start=(q == 0),
                        stop=(q == 1),
                    )
                nc.vector.tensor_relu(g2[:], pm[:, :bt])
                return g2

            def mlp3_out(g2, ti):
                # mlp3: (bt,128)->(bt,1)
                pm = ps.tile([1, 512], dt.float32, tag="ps")
                nc.tensor.matmul(pm[:, :bt], m3_sb[:], g2[:], start=True, stop=True)
                y_sb = sb.tile([1, bt], dt.float32, tag="y_sb", bufs=2)
                nc.vector.tensor_copy(y_sb[:], pm[:, :bt])
                nc.sync.dma_start(y_d[:, ti * bt : (ti + 1) * bt], y_sb[:])

            # Tile 0 runs straight through; tiles 1..NT-1 carry the previous
            # tile's mlp2/mlp3 inside their streams so those drain chains
            # (PSUM -> DVE relu -> PE) hide under matmul work.
            g1_prev = None
            prev_ti = None
            for ti in range(NT):
                h1 = conv1(ti)
                h2 = sb.tile([128, 14 * bt], dt.bfloat16, tag="h2")
                if g1_prev is not None:
                    g2_prev = mlp2(g1_prev)
                    conv2(h1, 0, 6)
                    mlp3_out(g2_prev, prev_ti)
                    conv2(h1, 6, 14)
                else:
                    conv2(h1, 0, 14)
                g1_prev = conv3_pool_mlp1(ti)
                prev_ti = ti

            # final tile: run the mlp tail in two 128-sample halves so the
            # PSUM->relu->matmul->copy drain chains pipeline instead of
            # serializing at the very end of the kernel
            g2 = sb.tile([128, bt], dt.bfloat16, tag="g2", bufs=2)
            y_sb = sb.tile([1, bt], dt.float32, tag="y_sb", bufs=2)
            pm2h = [
                ps.tile([128, 512], dt.float32, tag="ps", name=f"pm2h_{h}")
                for h in range(2)
            ]
            pm3h = [
                ps.tile([1, 512], dt.float32, tag="ps", name=f"pm3h_{h}")
                for h in range(2)
            ]
            hb = bt // 2
            for h in range(2):
                hs = slice(h * hb, (h + 1) * hb)
                for q in range(2):
                    nc.tensor.matmul(
                        pm2h[h][:, :hb],
                        m2_sb[:, q * 128 : (q + 1) * 128],
                        g1_prev[q][:, hs],
                        start=(q == 0),
                        stop=(q == 1),
                    )
                nc.vector.tensor_relu(g2[:, hs], pm2h[h][:, :hb])
            for h in range(2):
                hs = slice(h * hb, (h + 1) * hb)
                nc.tensor.matmul(
                    pm3h[h][:, :hb], m3_sb[:], g2[:, hs], start=True, stop=True
                )
                nc.vector.tensor_copy(y_sb[:, hs], pm3h[h][:, :hb])
            nc.sync.dma_start(y_d[:, prev_ti * bt : (prev_ti + 1) * bt], y_sb[:])

    nc.compile()
    return nc


def _prep_inputs(x, kernel_1, kernel_2, kernel_3, mlp_weight_1, mlp_weight_2, mlp_weight_3):
    """Host-side sharding + layout prep. Returns in_maps (one dict per core)."""
    # w1 chunks stacked along free dim: (128, q*192 + k*64 + co)
    w1f = kernel_1.transpose(1, 2, 0).reshape(512, 3 * 64)  # (cin, k*64+co)
    w1 = np.ascontiguousarray(
        w1f.reshape(4, 128, 192).transpose(1, 0, 2).reshape(128, 4 * 192)
    ).astype(BF16)
    # conv2 tap-pair blocks for the parity-split h1 layout: column block j is
    # a (128, 128) lhsT whose rows 0-63 multiply h1's even half and rows
    # 64-127 the odd half. Blocks 0-2 serve even output positions
    # ([k0;k1] [k2;k3] [k4;0]), blocks 3-5 odd ones ([0;k0] [k1;k2] [k3;k4]).
    k2t = kernel_2.transpose(1, 2, 0).astype(np.float32)  # (64, 5, 128)
    z = np.zeros((64, 128), np.float32)
    blocks = [
        np.concatenate([k2t[:, 0], k2t[:, 1]], axis=0),
        np.concatenate([k2t[:, 2], k2t[:, 3]], axis=0),
        np.concatenate([k2t[:, 4], z], axis=0),
        np.concatenate([z, k2t[:, 0]], axis=0),
        np.concatenate([k2t[:, 1], k2t[:, 2]], axis=0),
        np.concatenate([k2t[:, 3], k2t[:, 4]], axis=0),
    ]
    w2 = np.ascontiguousarray(np.concatenate(blocks, axis=1)).astype(BF16)
    w3 = np.ascontiguousarray(
        kernel_3.transpose(1, 2, 0).reshape(128, 7 * 256)
    ).astype(BF16)
    # W1 row f = c*4 + wp -> m1 row wp*256+c, then 8 chunks of 128 stacked
    # along the free dim ordered (wp*2 + q)
    m1r = mlp_weight_1.reshape(256, 4, 256).transpose(1, 0, 2).reshape(1024, 256)
    m1 = np.ascontiguousarray(
        m1r.reshape(8, 128, 256).transpose(1, 0, 2).reshape(128, 8 * 256)
    ).astype(BF16)
    m2 = np.ascontiguousarray(
        mlp_weight_2.reshape(2, 128, 128).transpose(1, 0, 2).reshape(128, 256)
    ).astype(BF16)
    m3 = mlp_weight_3.astype(BF16)

    xb = x.astype(BF16)
    in_maps = []
    for c in range(N_CORES):
        xc = xb[c * BC : (c + 1) * BC].transpose(1, 2, 0)  # (512, 20, BC)
        # tile-contiguous: per channel row, [tile][w][b] so each (tile,
        # c-chunk) DMA reads one contiguous 20*BT*2-byte run per partition.
        xt = np.ascontiguousarray(
            xc.reshape(512, 20, NT, BT).transpose(0, 2, 1, 3).reshape(512, NT * 20 * BT)
        )
        in_maps.append(
            {"x": xt, "w1": w1, "w2": w2, "w3": w3, "m1": m1, "m2": m2, "m3": m3}
        )
    return in_maps


def run(inputs, trace=False, **kw):
    """Compile (cached), run on 8 cores, return (y_full, BassKernelResults)."""
    from concourse import bass_utils

    if "nc" not in _compiled:
        _compiled["nc"] = _build()
    nc = _compiled["nc"]
    in_maps = _prep_inputs(**inputs)
    res = bass_utils.run_bass_kernel_spmd(
        nc, in_maps, core_ids=list(range(N_CORES)), trace=trace, **kw
    )
    y = np.concatenate(
        [res.results[c]["y"].reshape(BC, 1) for c in range(N_CORES)], axis=0
    )
    return y.astype(np.float32), res


def kernel(**inputs):
    inputs = {k: np.asarray(v) for k, v in inputs.items()}
    y, _ = run(inputs)
    return y


if __name__ == "__main__":
    rng = np.random.default_rng(0)
    inputs = {
        "x": rng.standard_normal((B, E, W), dtype=np.float32),
        "kernel_1": rng.standard_normal((64, 512, 3), dtype=np.float32),
        "kernel_2": rng.standard_normal((128, 64, 5), dtype=np.float32),
        "kernel_3": rng.standard_normal((256, 128, 7), dtype=np.float32),
        "mlp_weight_1": rng.standard_normal((1024, 256), dtype=np.float32),
        "mlp_weight_2": rng.standard_normal((256, 128), dtype=np.float32),
        "mlp_weight_3": rng.standard_normal((128, 1), dtype=np.float32),
    }
    y = kernel(**inputs)
    print("out", y.shape, y.dtype, y[:4, 0])


# revision 14
# speedup vs baseline: 1.1967x; 1.1967x over previous
"""Trainium2 Bass kernel for nn_CNNFromScratch (dense 1-D CNN + MLP head).

Strategy
--------
Pure data parallelism: the batch axis (8192) is split across 8 NeuronCores
(1024 samples each); conv kernels and MLP weights are replicated.

Per core, everything is expressed as TensorE matmuls with the contraction
(input channels x taps) on the partition axis:

  - x is pre-transposed on host to (C=512, tile, W=20, bt) and cast to bf16,
    so each (tile, c-chunk) loads as one DMA whose per-partition run is a
    single 20*bt*2-byte contiguous block (full HBM streaming bandwidth).
  - conv_k == sum over taps of  W_tap^T @ x[:, :, w+tap]  accumulated in PSUM.
  - Activations stay on-chip (SBUF, bf16) between layers; layout is
    (C_out partitions, w-major * batch free).
  - conv3's maxpool is fused from PSUM: relu(max(a,b)) == max-then-relu, so
    the per-position h3 buffer and its DVE relus disappear.

Batch is processed in four 256-sample tiles. Tile 0's conv1 is DMA-paced
(chunk-outer order starts the PE on the first 128-channel chunk, with dummy
"clock keeper" matmuls filling the inter-chunk DMA waits so the PE p-state
stays at full frequency); later tiles overlap their x DMA under the previous
tile's conv2/conv3/mlp work. mlp2/mlp3 of tile t are emitted inside tile
t+1's instruction stream so their PSUM->DVE->PE drain chains hide under
real matmul work.

Bulk DMAs are dep-free and rely on per-ring FIFO order for sequential
completion at full bandwidth. Constraints found by measurement: a
dep-waiting dma_start head-of-line blocks its issuing engine's sequencer;
two concurrently-streaming rings share HBM bandwidth (delaying the urgent
early chunks); and the Scalar ring is contended by system traffic
(instruction fetch). So all bulk DMAs ride the private GpSimd ring, in
priority order; only the small w1 (needed first, for PE warmup) goes on
the earlier-starting Scalar ring.

Matmul inputs are bf16 (1 cycle/row on PE), accumulation is fp32 in PSUM.
bt=256 keeps every matmul stream (256 rows) longer than its shadowed
LDWEIGHTS (~214 cyc), so weight loads stay off the critical path.
"""

import sys

sys.path.insert(0, "/opt/trn_rl_repo")

import numpy as np
import ml_dtypes

N_CORES = 8
B, E, W = 8192, 512, 20
BC = B // N_CORES  # samples per core
NT = 4  # batch tiles per core
BT = BC // NT  # samples per tile (256)

BF16 = ml_dtypes.bfloat16

_compiled = {}


def _build():
    import concourse.bass as bass
    from concourse import bacc, mybir
    import concourse.tile as tile

    dt = mybir.dt
    AF = mybir.ActivationFunctionType

    nc = bacc.Bacc(
        "TRN2",
        target_bir_lowering=False,
        debug=False,
        enable_asserts=False,
        num_devices=N_CORES,
    )

    # host packs all multi-chunk weights into 128-partition layouts so each
    # weight tensor is a single DMA
    x_d = nc.dram_tensor("x", (E, NT * W * BT), dt.bfloat16, kind="ExternalInput").ap()
    w1_d = nc.dram_tensor("w1", (128, 4 * 192), dt.bfloat16, kind="ExternalInput").ap()
    w2_d = nc.dram_tensor("w2", (128, 6 * 128), dt.bfloat16, kind="ExternalInput").ap()
    w3_d = nc.dram_tensor("w3", (128, 7 * 256), dt.bfloat16, kind="ExternalInput").ap()
    m1_d = nc.dram_tensor("m1", (128, 8 * 256), dt.bfloat16, kind="ExternalInput").ap()
    m2_d = nc.dram_tensor("m2", (128, 2 * 128), dt.bfloat16, kind="ExternalInput").ap()
    m3_d = nc.dram_tensor("m3", (128, 1), dt.bfloat16, kind="ExternalInput").ap()
    y_d = nc.dram_tensor("y", (1, BC), dt.float32, kind="ExternalOutput").ap()

    with tile.TileContext(nc) as tc:
        with (
            tc.tile_pool(name="sb", bufs=1) as sb,
            tc.tile_pool(name="ps", bufs=8, space="PSUM") as ps,
        ):
            # Bulk DMAs are issued dep-free, in priority order, all from the
            # (otherwise idle) GpSimd engine onto one ring: per-queue FIFO
            # order makes completions sequential at full bandwidth, with no
            # sequencer head-of-line blocking (a dep-waiting dma_start stalls
            # every later instruction on its issuing engine's sequencer).
            def chain(bass_inst):
                return bass_inst

            # ---- weights (resident for the whole kernel) ----
            # w1 loads first: the PE warmup matmuls read it, so the clock
            # ramp starts as soon as the DMA rings are up.
            w1_sb = sb.tile([128, 4 * 192], dt.bfloat16, tag="w1")
            nc.scalar.dma_start(w1_sb[:], w1_d[:, :])

            # Warm the PE clock gate while x streams in (dummy matmuls on the
            # already-loaded w1 tile; results never read) and pull the ACT
            # Relu table load off the critical path.
            warm_ps = ps.tile([128, 512], dt.float32, tag="ps", name="warm_ps")

            def warm(n):
                for _ in range(n):
                    nc.tensor.matmul(
                        warm_ps[0:64, 0:192],
                        w1_sb[:, 0:64],
                        w1_sb[:, 0:192],
                        start=True,
                        stop=True,
                    )

            warm(38)
            warm_act = sb.tile([1, 1], dt.float32, tag="warm_act")
            nc.scalar.activation(warm_act[:], w1_sb[0:1, 0:1], AF.Relu)

            # x chunk DMAs, tile 0 first; bulk weights ride after them
            # (needed from ~conv2 of tile 0); later tiles stream behind.
            x_sb = {}  # (t, q) -> sbuf tile view

            def load_x_tile(ti):
                for q in range(4):
                    t = sb.tile(
                        [128, W * BT],
                        dt.bfloat16,
                        tag=f"x_{q}",
                        bufs=3,
                        name=f"x_{ti}_{q}",
                    )
                    chain(
                        nc.gpsimd.dma_start(
                            t[:],
                            x_d[
                                q * 128 : (q + 1) * 128,
                                ti * W * BT : (ti + 1) * W * BT,
                            ],
                        )
                    )
                    x_sb[(ti, q)] = t

            load_x_tile(0)

            w2_sb = sb.tile([128, 6 * 128], dt.bfloat16, tag="w2")
            chain(nc.gpsimd.dma_start(w2_sb[:], w2_d[:, :]))
            w3_sb = sb.tile([128, 7 * 256], dt.bfloat16, tag="w3")
            chain(nc.gpsimd.dma_start(w3_sb[:], w3_d[:, :]))
            m1_sb = sb.tile([128, 8 * 256], dt.bfloat16, tag="m1")
            chain(nc.gpsimd.dma_start(m1_sb[:], m1_d[:, :]))
            m2_sb = sb.tile([128, 2 * 128], dt.bfloat16, tag="m2")
            chain(nc.gpsimd.dma_start(m2_sb[:], m2_d[:, :]))
            m3_sb = sb.tile([128, 1], dt.bfloat16, tag="m3")
            chain(nc.gpsimd.dma_start(m3_sb[:], m3_d[:, :]))

            for ti in range(1, NT):
                load_x_tile(ti)

            # ---- per-batch-tile pipeline ----
            bt = BT

            def conv1(ti):
                # conv1: (bt,512,20) -> relu -> (bt,64,18)
                # Output positions are packed in pairs: even w on PSUM/SBUF
                # partitions 0-63, odd w on 64-127. The two M=64 accumulation
                # groups land on different PE column groups and execute
                # concurrently (~2x conv1 throughput). Chunk-outer order lets
                # each block start as soon as its c-chunk DMA lands.
                h1 = sb.tile([128, 9 * bt], dt.bfloat16, tag="h1")

                def mms(p1, u, q):
                    for k in range(3):
                        nc.tensor.matmul(
                            p1[0:64, :bt],
                            w1_sb[:, q * 192 + k * 64 : q * 192 + (k + 1) * 64],
                            x_sb[(ti, q)][:, (2 * u + k) * bt : (2 * u + k + 1) * bt],
                            start=(q == 0 and k == 0),
                            stop=(q == 3 and k == 2),
                            skip_group_check=True,
                        )
                        nc.tensor.matmul(
                            p1[64:128, :bt],
                            w1_sb[:, q * 192 + k * 64 : q * 192 + (k + 1) * 64],
                            x_sb[(ti, q)][
                                :, (2 * u + 1 + k) * bt : (2 * u + 2 + k) * bt
                            ],
                            start=(q == 0 and k == 0),
                            stop=(q == 3 and k == 2),
                            skip_group_check=True,
                        )

                # u-blocks of (7,2): at most 7 PSUM tiles live per block
                # (plus warm_ps's slot for u7), so the 8-slot ring never
                # aliases a tile that is still accumulating
                for u0, u1 in ((0, 7), (7, 9)):
                    p1s = [
                        ps.tile([128, 512], dt.float32, tag="ps", name=f"p1_{u}")
                        for u in range(u0, u1)
                    ]
                    for q in range(4):
                        for u in range(u0, u1):
                            mms(p1s[u - u0], u, q)
                    for u in range(u0, u1):
                        nc.scalar.activation(
                            h1[:, u * bt : (u + 1) * bt], p1s[u - u0][:, :bt], AF.Relu
                        )
                return h1

            def conv2(h1, wlo, whi):
                # conv2: -> relu -> (bt,128,14)
                # h1's parity-split layout lets adjacent taps fuse into one
                # full 128-row contraction: 3 matmuls per position.
                for w in range(wlo, whi):
                    t0 = w // 2
                    blk0 = 0 if w % 2 == 0 else 3
                    p2 = ps.tile([128, 512], dt.float32, tag="ps")
                    for j in range(3):
                        blk = blk0 + j
                        nc.tensor.matmul(
                            p2[:, :bt],
                            w2_sb[:, blk * 128 : (blk + 1) * 128],
                            h1[:, (t0 + j) * bt : (t0 + j + 1) * bt],
                            start=(j == 0),
                            stop=(j == 2),
                        )
                    nc.vector.tensor_relu(
                        h2[:, w * bt : (w + 1) * bt], p2[:, :bt]
                    )

            def conv3_pool_mlp1(ti):
                # conv3: -> (bt,256,8) as two 128-channel halves, pooled
                # straight out of PSUM: max(even,odd) on DVE, relu on ACT
                # (relu(max(a,b)) == max-then-relu).
                pooled = [
                    sb.tile(
                        [128, 4 * bt], dt.bfloat16, tag=f"pool_{m}", name=f"pool_{m}"
                    )
                    for m in range(2)
                ]
                ptmp = sb.tile([128, 4 * bt], dt.bfloat16, tag="ptmp")
                ptm2 = sb.tile([128, 4 * bt], dt.bfloat16, tag="ptm2")
                for m in range(2):
                    for p in range(4):
                        # even position drains via ACT relu (off the critical
                        # path, under the odd position's matmuls); DVE takes
                        # max(relu(even)_sbuf, odd_psum); final ACT relu
                        # completes relu(max(even, odd)).
                        pp = []
                        for w in (2 * p, 2 * p + 1):
                            p3 = ps.tile([128, 512], dt.float32, tag="ps")
                            for k in range(7):
                                nc.tensor.matmul(
                                    p3[:, :bt],
                                    w3_sb[
                                        :, k * 256 + m * 128 : k * 256 + (m + 1) * 128
                                    ],
                                    h2[:, (w + k) * bt : (w + k + 1) * bt],
                                    start=(k == 0),
                                    stop=(k == 6),
                                )
                            pp.append(p3)
                        ecol = ptmp[:, p * bt : (p + 1) * bt]
                        nc.scalar.activation(ecol, pp[0][:, :bt], AF.Relu)
                        mcol = ptm2[:, p * bt : (p + 1) * bt]
                        nc.vector.tensor_max(mcol, ecol, pp[1][:, :bt])
                        nc.scalar.activation(
                            pooled[m][:, p * bt : (p + 1) * bt], mcol, AF.Relu
                        )

                # mlp1: (bt,1024)->(bt,256), f = c*4 + wp. j/q interleaved so
                # the pooled[1]-dependent matmuls start ~8 matmuls after the
                # last conv3 position, covering its max+relu drain.
                g1 = [
                    sb.tile([128, bt], dt.bfloat16, tag=f"g1_{j}", bufs=2, name=f"g1_{j}")
                    for j in range(2)
                ]
                pms = [
                    ps.tile([128, 512], dt.float32, tag="ps", name=f"pm1_{j}")
                    for j in range(2)
                ]
                for q in range(2):
                    for j in range(2):
                        for wp in range(4):
                            nc.tensor.matmul(
                                pms[j][:, :bt],
                                m1_sb[:, (wp * 2 + q) * 256 + j * 128 : (wp * 2 + q) * 256 + (j + 1) * 128],
                                pooled[q][:, wp * bt : (wp + 1) * bt],
                                start=(wp == 0 and q == 0),
                                stop=(wp == 3 and q == 1),
                            )
                for j in range(2):
                    nc.vector.tensor_relu(g1[j][:], pms[j][:, :bt])
                return g1

            def mlp2(g1):
                # mlp2: (bt,256)->(bt,128)
                g2 = sb.tile([128, bt], dt.bfloat16, tag="g2", bufs=2)
                pm = ps.tile([128, 512], dt.float32, tag="ps")
                for q in range(2):
                    nc.tensor.matmul(
                        pm[:, :bt],
                        m2_sb[:, q * 128 : (q + 1) * 128],
                        g1[q][:],
                        start=(q == 0),
                        stop=(q == 1),
                    )
                nc.vector.tensor_relu(g2[:], pm[:, :bt])
                return g2

            def mlp3_out(g2, ti):
                # mlp3: (bt,128)->(bt,1)
                pm = ps.tile([1, 512], dt.float32, tag="ps")
                nc.tensor.matmul(pm[:, :bt], m3_sb[:], g2[:], start=True, stop=True)
                y_sb = sb.tile([1, bt], dt.float32, tag="y_sb", bufs=2)
                nc.vector.tensor_copy(y_sb[:], pm[:, :bt])
                nc.sync.dma_start(y_d[:, ti * bt : (ti + 1) * bt], y_sb[:])

            # Tile 0 runs straight through; tiles 1..NT-1 carry the previous
            # tile's mlp2/mlp3 inside their streams so those drain chains
            # (PSUM -> DVE relu -> PE) hide under matmul work.
            g1_prev = None
            prev_ti = None
            for ti in range(NT):
                h1 = conv1(ti)
                h2 = sb.tile([128, 14 * bt], dt.bfloat16, tag="h2")
                if g1_prev is not None:
                    g2_prev = mlp2(g1_prev)
                    conv2(h1, 0, 6)
                    mlp3_out(g2_prev, prev_ti)
                    conv2(h1, 6, 14)
                else:
                    conv2(h1, 0, 14)
                g1_prev = conv3_pool_mlp1(ti)
                prev_ti = ti

            # final tile: run the mlp tail in two 128-sample halves so the
            # PSUM->relu->matmul->copy drain chains pipeline instead of
            # serializing at the very end of the kernel
            g2 = sb.tile([128, bt], dt.bfloat16, tag="g2", bufs=2)
            y_sb = sb.tile([1, bt], dt.float32, tag="y_sb", bufs=2)
            pm2h = [
                ps.tile([128, 512], dt.float32, tag="ps", name=f"pm2h_{h}")
                for h in range(2)
            ]
            pm3h = [
                ps.tile([1, 512], dt.float32, tag="ps", name=f"pm3h_{h}")
                for h in range(2)
            ]
            hb = bt // 2
            for h in range(2):
                hs = slice(h * hb, (h + 1) * hb)
                for q in range(2):
                    nc.tensor.matmul(
                        pm2h[h][:, :hb],
                        m2_sb[:, q * 128 : (q + 1) * 128],
                        g1_prev[q][:, hs],
                        start=(q == 0),
                        stop=(q == 1),
                    )
                nc.vector.tensor_relu(g2[:, hs], pm2h[h][:, :hb])
            for h in range(2):
                hs = slice(h * hb, (h + 1) * hb)
                nc.tensor.matmul(
                    pm3h[h][:, :hb], m3_sb[:], g2[:, hs], start=True, stop=True
                )
                nc.vector.tensor_copy(y_sb[:, hs], pm3h[h][:, :hb])
            nc.sync.dma_start(y_d[:, prev_ti * bt : (prev_ti + 1) * bt], y_sb[:])

    nc.compile()
    return nc


def _prep_inputs(x, kernel_1, kernel_2, kernel_3, mlp_weight_1, mlp_weight_2, mlp_weight_3):
    """Host-side sharding + layout prep. Returns in_maps (one dict per core)."""
    # w1 chunks stacked along free dim: (128, q*192 + k*64 + co)
    w1f = kernel_1.transpose(1, 2, 0).reshape(512, 3 * 64)  # (cin, k*64+co)
    w1 = np.ascontiguousarray(
        w1f.reshape(4, 128, 192).transpose(1, 0, 2).reshape(128, 4 * 192)
    ).astype(BF16)
    # conv2 tap-pair blocks for the parity-split h1 layout: column block j is
    # a (128, 128) lhsT whose rows 0-63 multiply h1's even half and rows
    # 64-127 the odd half. Blocks 0-2 serve even output positions
    # ([k0;k1] [k2;k3] [k4;0]), blocks 3-5 odd ones ([0;k0] [k1;k2] [k3;k4]).
    k2t = kernel_2.transpose(1, 2, 0).astype(np.float32)  # (64, 5, 128)
    z = np.zeros((64, 128), np.float32)
    blocks = [
        np.concatenate([k2t[:, 0], k2t[:, 1]], axis=0),
        np.concatenate([k2t[:, 2], k2t[:, 3]], axis=0),
        np.concatenate([k2t[:, 4], z], axis=0),
        np.concatenate([z, k2t[:, 0]], axis=0),
        np.concatenate([k2t[:, 1], k2t[:, 2]], axis=0),
        np.concatenate([k2t[:, 3], k2t[:, 4]], axis=0),
    ]
    w2 = np.ascontiguousarray(np.concatenate(blocks, axis=1)).astype(BF16)
    w3 = np.ascontiguousarray(
        kernel_3.transpose(1, 2, 0).reshape(128, 7 * 256)
    ).astype(BF16)
    # W1 row f = c*4 + wp -> m1 row wp*256+c, then 8 chunks of 128 stacked
    # along the free dim ordered (wp*2 + q)
    m1r = mlp_weight_1.reshape(256, 4, 256).transpose(1, 0, 2).reshape(1024, 256)
    m1 = np.ascontiguousarray(
        m1r.reshape(8, 128, 256).transpose(1, 0, 2).reshape(128, 8 * 256)
    ).astype(BF16)
    m2 = np.ascontiguousarray(
        mlp_weight_2.reshape(2, 128, 128).transpose(1, 0, 2).reshape(128, 256)
    ).astype(BF16)
    m3 = mlp_weight_3.astype(BF16)

    xb = x.astype(BF16)
    in_maps = []
    for c in range(N_CORES):
        xc = xb[c * BC : (c + 1) * BC].transpose(1, 2, 0)  # (512, 20, BC)
        # tile-contiguous: per channel row, [tile][w][b] so each (tile,
        # c-chunk) DMA reads one contiguous 20*BT*2-byte run per partition.
        xt = np.ascontiguousarray(
            xc.reshape(512, 20, NT, BT).transpose(0, 2, 1, 3).reshape(512, NT * 20 * BT)
        )
        in_maps.append(
            {"x": xt, "w1": w1, "w2": w2, "w3": w3, "m1": m1, "m2": m2, "m3": m3}
        )
    return in_maps


def run(inputs, trace=False, **kw):
    """Compile (cached), run on 8 cores, return (y_full, BassKernelResults)."""
    from concourse import bass_utils

    if "nc" not in _compiled:
        _compiled["nc"] = _build()
    nc = _compiled["nc"]
    in_maps = _prep_inputs(**inputs)
    res = bass_utils.run_bass_kernel_spmd(
        nc, in_maps, core_ids=list(range(N_CORES)), trace=trace, **kw
    )
    y = np.concatenate(
        [res.results[c]["y"].reshape(BC, 1) for c in range(N_CORES)], axis=0
    )
    return y.astype(np.float32), res


def kernel(**inputs):
    inputs = {k: np.asarray(v) for k, v in inputs.items()}
    y, _ = run(inputs)
    return y


if __name__ == "__main__":
    rng = np.random.default_rng(0)
    inputs = {
        "x": rng.standard_normal((B, E, W), dtype=np.float32),
        "kernel_1": rng.standard_normal((64, 512, 3), dtype=np.float32),
        "kernel_2": rng.standard_normal((128, 64, 5), dtype=np.float32),
        "kernel_3": rng.standard_normal((256, 128, 7), dtype=np.float32),
        "mlp_weight_1": rng.standard_normal((1024, 256), dtype=np.float32),
        "mlp_weight_2": rng.standard_normal((256, 128), dtype=np.float32),
        "mlp_weight_3": rng.standard_normal((128, 1), dtype=np.float32),
    }
    y = kernel(**inputs)
    print("out", y.shape, y.dtype, y[:4, 0])
